# revision 18
# baseline (speedup 1.0000x reference)
"""Trainium2 Bass kernel for CFContrastiveLoss.

Reference semantics (per sample of N=16 options, D=768 dims):
  - L2-normalize option embeddings
  - sim = pairwise cosine sims within the sample (16x16 gram)
  - max_neg[n] = max over negative-labeled columns of sim[n, :]
  - loss = mean over (positive rows of valid samples) of relu(max_neg + 0.3)

Device strategy (pure data parallel over batch, 8 cores):
  - 128 rows (= 8 samples x 16 options) per "group"; per core 16384 rows
    = 128 groups.  Each super-group is ONE whole DMA transfer, the two
    HWDGE rings taking alternate super-groups (a ring issues only every
    other sg, so per-transfer issue latency and completion waits never
    starve the stream); the schedule is tapered (4,4,8,16,...,16,8,4,4)
    so the first PSUM bank's compute starts early and the post-last-byte
    compute tail is only 4 groups.  Mask loads ride the opposite ring
    from their sg's embedding transfer, behind it, so they cannot
    head-of-line block the stream.
  - Host pre-normalizes embeddings, scales by 16 (power of two; keeps
    elements inside e4m3's normal range) and casts to fp8 e4m3 in the
    matmul layout.  This is a memory-bound problem, so fp8 halves the
    HBM traffic vs fp16.  Per-sample gram matrices are computed on the
    TensorEngine as block-diagonal 128x128 grams (fp32 PSUM accumulate)
    using DoubleRow fp8 matmuls: each matmul consumes TWO 128-row
    k-subtiles at the double-pumped fp8 rate, so the 768-dim contraction
    is 3 matmuls instead of 6.  Sims come out scaled by 256; the host
    divides it back out.  e4m3 carries 3 mantissa bits; the per-sim
    error (~2e-3 absolute) averages out over the ~52k contributing rows
    and the max() bias stays small because top-sim gaps are larger than
    the noise.  Measured final loss error ~1.4e-4 (threshold 2e-2).
  - The label/validity masking is folded into the same PSUM accumulation
    as sentinel outer-product matmuls of +-2^14 (every factor is +-128
    or 0, all fp8-exact):
      row 0:  128*ones x negc       (negc[m] = -128 iff label[m] == 1)
      row 1:  128*ones x (-128*ones)  (mask everything ...)
      row 2+s: 128*u_s x (128*u_s)    (... except within-sample blocks)
    Sentinels are exact powers of two so they cancel exactly in fp32:
    in-block negative columns get an exactly-zero mask contribution and
    unmasked sims are bit-exact; masked entries sit at <= -2^14 + 256 so
    relu(max/256 + margin) = 0.  One 512-column normal-mode (FWL) mask
    matmul per PSUM bank covers 4 groups; it runs LAST in the bank's
    accumulation so the bank's gram compute can begin before the mask
    operands have landed.  start=True marks the WHOLE bank pending-zero
    (per-element has_written bits), so only the bank's first gram
    matmul sets it; the mask and later groups accumulate/overwrite via
    the has_written bits.  Mask rhs rows MASK_K..127 are memset to zero
    once so the zero lhsT pad rows never multiply NaN garbage.
  - Per bank the device then does a single VectorE row-max from PSUM
    over a [128, 4, 128] view (reduces the innermost axis -> [128, 4]),
    amortizing DVE instruction overhead over 4 groups.  All row maxes
    accumulate in one [128, 128] SBUF tile, stored with a single DMA at
    the end.  relu/margin/weighting/final mean are O(rows) on host.
"""

import os

import ml_dtypes
import numpy as np

import concourse.bass as bass
import concourse.mybir as mybir
from concourse import bacc, tile
from concourse.bass_utils import run_bass_kernel_spmd

FP8 = mybir.dt.float8e4
NP_FP8 = ml_dtypes.float8_e4m3
F32 = mybir.dt.float32
DOUBLE_ROW = mybir.MatmulPerfMode.DoubleRow

B, N, D = 8192, 16, 768
N_CORES = 8
ROWS = B * N                      # 131072
ROWS_PER_CORE = ROWS // N_CORES   # 16384
GROUPS = ROWS_PER_CORE // 128     # 128 groups of 128 rows per core
KCH = D // 128                    # 6 contraction chunks
SG_MAX = 16                       # largest super-group (SBUF tile size)
GPB = 4                           # groups per PSUM bank (512 f32 / 128)
# Tapered schedule: fast ramp-in, long efficient middle, short tail.
SCHED = [4, 4, 8, 16, 16, 16, 16, 16, 16, 8, 4, 4]
assert sum(SCHED) == GROUPS
MASK_K = 2 + 128 // N             # 10 live mask matmul rows
SENT = np.float32(128.0)          # fp8-exact sentinel factor (128*128 = 2^14)
ESCALE = np.float32(16.0)         # fp8 embedding scale (power of two)
SIM_SCALE = ESCALE * ESCALE       # gram outputs are scaled by this
MARGIN = np.float32(0.3)

_CACHE: dict = {}

LAST_RESULT = None  # BassKernelResults of the most recent device run


def _build_program() -> bass.Bass:
    nc = bacc.Bacc(None)
    et = nc.declare_dram_parameter("et", [128, GROUPS * KCH, 128], FP8, isOutput=False)
    mrhs = nc.declare_dram_parameter("mrhs", [MASK_K, GROUPS * 128], FP8, isOutput=False)
    mlhs = nc.declare_dram_parameter("mlhs", [128, 128], FP8, isOutput=False)
    out = nc.declare_dram_parameter("out", [128, GROUPS], F32, isOutput=True)

    with tile.TileContext(nc) as tc:
        with (
            tc.tile_pool(name="emb", bufs=6) as emb_pool,
            tc.tile_pool(name="const", bufs=1) as const_pool,
            tc.tile_pool(name="psum", bufs=8, space="PSUM") as psum_pool,
        ):
            # Mask lhsT: a plain [128, 128] weight tensor (normal-mode
            # matmul -> automatic Fast Weight Load); rows MASK_K..127 are
            # zero in dram already.
            mlhs_t = const_pool.tile([128, 128], FP8)
            nc.scalar.dma_start(mlhs_t[:, :], mlhs[:])
            # Four rotating mask-rhs tiles; rows (MASK_K..127) stay zero
            # -- memset once (split over two engines, concurrently; off
            # the critical path of the embedding stream) so the padded
            # lanes contribute nothing (and no NaN*0).
            mr_tiles = []
            for i in range(4):
                mr_t = const_pool.tile([128, SG_MAX * 128], FP8, name=f"mr{i}")
                eng = nc.vector if i % 2 == 0 else nc.gpsimd
                eng.memset(mr_t[:, :], 0.0)
                mr_tiles.append(mr_t)
            # All 128 row-maxes accumulate here; one DMA at the end.
            wide = const_pool.tile([128, GROUPS], F32)

            g0 = 0
            for sg, ng in enumerate(SCHED):
                hi = emb_pool.tile([128, SG_MAX * KCH, 128], FP8, tag="hi")
                mr_t = mr_tiles[sg % 4]
                nk = ng * KCH
                c0 = g0 * KCH
                # One whole-super-group transfer, alternating rings:
                # each ring issues only every other sg, so queue issue
                # latency and completion waits never starve the stream.
                # The mask load rides the OPPOSITE ring.
                ring = nc.sync if sg % 2 == 0 else nc.scalar
                oring = nc.scalar if sg % 2 == 0 else nc.sync
                ring.dma_start(hi[:, :nk, :], et[:, c0:c0 + nk, :])
                oring.dma_start(
                    mr_t[:MASK_K, :ng * 128],
                    mrhs[:, g0 * 128:(g0 + ng) * 128])
                for b in range(ng // GPB):
                    ps = psum_pool.tile([128, GPB, 128], F32)  # one full PSUM bank
                    for g in range(GPB):
                        gi = b * GPB + g
                        G = ps[:, g, :]
                        for k in range(KCH // 2):
                            kc = gi * KCH + 2 * k
                            hk = hi[:, kc:kc + 2, :]
                            # start=True marks the WHOLE bank pending-zero
                            # (per-element has_written bits), so only the
                            # bank's first matmul sets it; later groups'
                            # first writes land on pending-zero bytes and
                            # overwrite, everything else accumulates.
                            nc.tensor.matmul(
                                G, hk, hk,
                                start=(g == 0 and k == 0), stop=False,
                                perf_mode=DOUBLE_ROW,
                            )
                    # Mask sentinels for the whole bank, accumulated LAST
                    # (normal mode: gets FWL, no DoubleRow LDW penalty).
                    mc = b * GPB * 128
                    nc.tensor.matmul(
                        ps[:, :, :], mlhs_t[:, :], mr_t[:, mc:mc + GPB * 128],
                        start=False, stop=True,
                    )
                    # One DVE reduce for the whole bank: [128, 4, 128]
                    # reduced over the innermost axis -> [128, 4].
                    nc.vector.reduce_max(
                        wide[:, g0 + b * GPB: g0 + (b + 1) * GPB],
                        ps[:, :, :], axis=mybir.AxisListType.X)
                g0 += ng
            nc.sync.dma_start(out[:, :], wide[:, :])
    nc.finalize()
    return nc


def _prep_core_inputs(Xn8: np.ndarray, lab: np.ndarray):
    """Per-core input maps from fp8-normalized embeddings + flat labels."""
    negc = np.where(lab == 1, -SENT, np.float32(0.0)).astype(NP_FP8)

    m_idx = np.arange(128)
    # mask lhsT: [128, 128], rows = SENT * [ones; ones; u_0..u_7; zeros...]
    mlhs = np.zeros((128, 128), dtype=NP_FP8)
    mlhs[0, :] = SENT
    mlhs[1, :] = SENT
    for s in range(128 // N):
        mlhs[2 + s, :] = (SENT * (m_idx // N == s)).astype(NP_FP8)

    # static part of mask rhs rows 1..9 (per 128-column group)
    mrhs_static = np.zeros((MASK_K, 128), dtype=NP_FP8)
    mrhs_static[1, :] = -SENT
    for s in range(128 // N):
        mrhs_static[2 + s, :] = np.where(m_idx // N == s, SENT, np.float32(0.0)).astype(NP_FP8)

    def to_layout(a_core: np.ndarray) -> np.ndarray:
        # [rows=16384, 768] -> [g, n, k, p] -> [p, g, k, n]
        return np.ascontiguousarray(
            a_core.reshape(GROUPS, 128, KCH, 128).transpose(3, 0, 2, 1)
        ).reshape(128, GROUPS * KCH, 128)

    in_maps = []
    for c in range(N_CORES):
        r0 = c * ROWS_PER_CORE
        r1 = r0 + ROWS_PER_CORE
        mr = np.empty((MASK_K, ROWS_PER_CORE), dtype=NP_FP8)
        mr[0, :] = negc[r0:r1]
        mr[1:, :] = np.tile(mrhs_static[1:, :], (1, GROUPS))
        in_maps.append({
            "et": to_layout(Xn8[r0:r1]),
            "mrhs": mr,
            "mlhs": mlhs,
        })
    return in_maps


def kernel(embeddings: np.ndarray, labels: np.ndarray) -> np.ndarray:
    global LAST_RESULT
    assert embeddings.shape == (B, N, D)
    assert labels.shape == (B, N)

    X = np.asarray(embeddings, dtype=np.float32).reshape(ROWS, D)
    lab = np.asarray(labels).reshape(ROWS)

    ss = np.square(X).sum(axis=1, dtype=np.float32)
    norms = np.sqrt(ss)
    Xn8 = (X * (ESCALE / np.maximum(norms, np.float32(1e-12)))[:, None]).astype(NP_FP8)

    in_maps = _prep_core_inputs(Xn8, lab)

    if "nc" not in _CACHE:
        _CACHE["nc"] = _build_program()
    nc = _CACHE["nc"]

    trace = os.environ.get("BASS_KERNEL_TRACE", "0") == "1"
    res = run_bass_kernel_spmd(nc, in_maps, list(range(N_CORES)), trace=trace)
    LAST_RESULT = res

    # out[p, g]: group-major row r = g*128 + p
    maxneg = np.concatenate(
        [np.asarray(r["out"]).T.reshape(ROWS_PER_CORE) for r in res.results]
    )

    triplet = np.maximum(maxneg / SIM_SCALE + MARGIN, np.float32(0.0))
    has_neg = (np.asarray(labels) == 0).any(axis=1)
    w = (lab == 1) & np.repeat(has_neg, N)
    loss_sum = np.float32((triplet * w).sum(dtype=np.float64))
    count = int(w.sum())
    loss = np.float32(loss_sum / np.float32(max(count, 1)))
    return np.asarray(loss, dtype=np.float32)


# revision 19
# speedup vs baseline: 1.0938x; 1.0938x over previous
"""Trainium2 Bass kernel for CFContrastiveLoss.

Reference semantics (per sample of N=16 options, D=768 dims):
  - L2-normalize option embeddings
  - sim = pairwise cosine sims within the sample (16x16 gram)
  - max_neg[n] = max over negative-labeled columns of sim[n, :]
  - loss = mean over (positive rows of valid samples) of relu(max_neg + 0.3)

Device strategy (pure data parallel over batch, 8 cores):
  - 128 rows (= 8 samples x 16 options) per "group"; per core 16384 rows
    = 128 groups.  Each super-group is ONE whole DMA transfer, the two
    HWDGE rings taking alternate super-groups (a ring issues only every
    other sg, so per-transfer issue latency and completion waits never
    starve the stream); the schedule is tapered (4,4,8,16,...,16,8,4,4)
    so the first PSUM bank's compute starts early and the post-last-byte
    compute tail is only 4 groups.  Mask loads ride the opposite ring
    from their sg's embedding transfer, behind it, so they cannot
    head-of-line block the stream.
  - Host pre-normalizes embeddings, scales by 16 (power of two; keeps
    elements inside e4m3's normal range) and casts to fp8 e4m3 in the
    matmul layout.  This is a memory-bound problem, so fp8 halves the
    HBM traffic vs fp16.  Per-sample gram matrices are computed on the
    TensorEngine as block-diagonal 128x128 grams (fp32 PSUM accumulate)
    using DoubleRow fp8 matmuls: each matmul consumes TWO 128-row
    k-subtiles at the double-pumped fp8 rate, so the 768-dim contraction
    is 3 matmuls instead of 6.  Sims come out scaled by 256; the host
    divides it back out.  e4m3 carries 3 mantissa bits; the per-sim
    error (~2e-3 absolute) averages out over the ~52k contributing rows
    and the max() bias stays small because top-sim gaps are larger than
    the noise.  Measured final loss error ~1.4e-4 (threshold 2e-2).
  - The label/validity masking is folded into the same PSUM accumulation
    as sentinel outer-product matmuls of +-2^14 (every factor is +-128
    or 0, all fp8-exact):
      row 0:  128*ones x negc       (negc[m] = -128 iff label[m] == 1)
      row 1:  128*ones x (-128*ones)  (mask everything ...)
      row 2+s: 128*u_s x (128*u_s)    (... except within-sample blocks)
    Sentinels are exact powers of two so they cancel exactly in fp32:
    in-block negative columns get an exactly-zero mask contribution and
    unmasked sims are bit-exact; masked entries sit at <= -2^14 + 256 so
    relu(max/256 + margin) = 0.  One 512-column normal-mode (FWL) mask
    matmul per PSUM bank covers 4 groups; it runs LAST in the bank's
    accumulation so the bank's gram compute can begin before the mask
    operands have landed.  start=True marks the WHOLE bank pending-zero
    (per-element has_written bits), so only the bank's first gram
    matmul sets it; the mask and later groups accumulate/overwrite via
    the has_written bits.  Mask rhs rows MASK_K..127 are memset to zero
    once so the zero lhsT pad rows never multiply NaN garbage.
  - Per bank the device then does a single VectorE row-max from PSUM
    over a [128, 4, 128] view (reduces the innermost axis -> [128, 4]),
    amortizing DVE instruction overhead over 4 groups.  All row maxes
    accumulate in one [128, 128] SBUF tile, stored with a single DMA at
    the end.  relu/margin/weighting/final mean are O(rows) on host.
"""

import os

import ml_dtypes
import numpy as np

import concourse.bass as bass
import concourse.mybir as mybir
from concourse import bacc, tile
from concourse.bass_utils import run_bass_kernel_spmd

FP8 = mybir.dt.float8e4
NP_FP8 = ml_dtypes.float8_e4m3
F32 = mybir.dt.float32
DOUBLE_ROW = mybir.MatmulPerfMode.DoubleRow

B, N, D = 8192, 16, 768
N_CORES = 8
ROWS = B * N                      # 131072
ROWS_PER_CORE = ROWS // N_CORES   # 16384
GROUPS = ROWS_PER_CORE // 128     # 128 groups of 128 rows per core
KCH = D // 128                    # 6 contraction chunks
SG_MAX = 16                       # largest super-group (SBUF tile size)
GPB = 4                           # groups per PSUM bank (512 f32 / 128)
# Tapered schedule: fast ramp-in, long efficient middle, short tail.
SCHED = [4, 4, 8, 16, 16, 16, 16, 16, 16, 8, 4, 4]
assert sum(SCHED) == GROUPS
MASK_K = 2 + 128 // N             # 10 live mask matmul rows
SENT = np.float32(128.0)          # fp8-exact sentinel factor (128*128 = 2^14)
ESCALE = np.float32(16.0)         # fp8 embedding scale (power of two)
SIM_SCALE = ESCALE * ESCALE       # gram outputs are scaled by this
MARGIN = np.float32(0.3)

_CACHE: dict = {}

LAST_RESULT = None  # BassKernelResults of the most recent device run


def _build_program() -> bass.Bass:
    nc = bacc.Bacc(None)
    et = nc.declare_dram_parameter("et", [128, GROUPS * KCH, 128], FP8, isOutput=False)
    mrhs = nc.declare_dram_parameter("mrhs", [MASK_K, GROUPS * 128], FP8, isOutput=False)
    mlhs = nc.declare_dram_parameter("mlhs", [128, 128], FP8, isOutput=False)
    out = nc.declare_dram_parameter("out", [128, GROUPS], F32, isOutput=True)

    with tile.TileContext(nc) as tc:
        with (
            tc.tile_pool(name="emb", bufs=8) as emb_pool,
            tc.tile_pool(name="const", bufs=1) as const_pool,
            tc.tile_pool(name="psum", bufs=8, space="PSUM") as psum_pool,
        ):
            # Mask lhsT: a plain [128, 128] weight tensor (normal-mode
            # matmul -> automatic Fast Weight Load); rows MASK_K..127 are
            # zero in dram already.
            mlhs_t = const_pool.tile([128, 128], FP8)
            nc.scalar.dma_start(mlhs_t[:, :], mlhs[:])
            # Four rotating mask-rhs tiles; rows (MASK_K..127) stay zero
            # -- memset once (split over two engines, concurrently; off
            # the critical path of the embedding stream) so the padded
            # lanes contribute nothing (and no NaN*0).
            mr_tiles = []
            for i in range(4):
                mr_t = const_pool.tile([128, SG_MAX * 128], FP8, name=f"mr{i}")
                eng = nc.vector if i % 2 == 0 else nc.gpsimd
                eng.memset(mr_t[:, :], 0.0)
                mr_tiles.append(mr_t)
            # All 128 row-maxes accumulate here; one DMA at the end.
            wide = const_pool.tile([128, GROUPS], F32)

            g0 = 0
            for sg, ng in enumerate(SCHED):
                hi = emb_pool.tile([128, SG_MAX * KCH, 128], FP8, tag="hi")
                mr_t = mr_tiles[sg % 4]
                nk = ng * KCH
                c0 = g0 * KCH
                # One whole-super-group transfer, alternating rings:
                # each ring issues only every other sg, so queue issue
                # latency and completion waits never starve the stream.
                # The mask load rides the OPPOSITE ring.
                ring = nc.sync if sg % 2 == 0 else nc.scalar
                oring = nc.scalar if sg % 2 == 0 else nc.sync
                ring.dma_start(hi[:, :nk, :], et[:, c0:c0 + nk, :])
                oring.dma_start(
                    mr_t[:MASK_K, :ng * 128],
                    mrhs[:, g0 * 128:(g0 + ng) * 128])
                for b in range(ng // GPB):
                    ps = psum_pool.tile([128, GPB, 128], F32)  # one full PSUM bank
                    for g in range(GPB):
                        gi = b * GPB + g
                        G = ps[:, g, :]
                        for k in range(KCH // 2):
                            kc = gi * KCH + 2 * k
                            hk = hi[:, kc:kc + 2, :]
                            # start=True marks the WHOLE bank pending-zero
                            # (per-element has_written bits), so only the
                            # bank's first matmul sets it; later groups'
                            # first writes land on pending-zero bytes and
                            # overwrite, everything else accumulates.
                            nc.tensor.matmul(
                                G, hk, hk,
                                start=(g == 0 and k == 0), stop=False,
                                perf_mode=DOUBLE_ROW,
                            )
                    # Mask sentinels for the whole bank, accumulated LAST
                    # (normal mode: gets FWL, no DoubleRow LDW penalty).
                    mc = b * GPB * 128
                    nc.tensor.matmul(
                        ps[:, :, :], mlhs_t[:, :], mr_t[:, mc:mc + GPB * 128],
                        start=False, stop=True,
                    )
                    # One DVE reduce for the whole bank: [128, 4, 128]
                    # reduced over the innermost axis -> [128, 4].
                    nc.vector.reduce_max(
                        wide[:, g0 + b * GPB: g0 + (b + 1) * GPB],
                        ps[:, :, :], axis=mybir.AxisListType.X)
                g0 += ng
            nc.sync.dma_start(out[:, :], wide[:, :])
    nc.finalize()
    return nc


def _prep_core_inputs(Xn8: np.ndarray, lab: np.ndarray):
    """Per-core input maps from fp8-normalized embeddings + flat labels."""
    negc = np.where(lab == 1, -SENT, np.float32(0.0)).astype(NP_FP8)

    m_idx = np.arange(128)
    # mask lhsT: [128, 128], rows = SENT * [ones; ones; u_0..u_7; zeros...]
    mlhs = np.zeros((128, 128), dtype=NP_FP8)
    mlhs[0, :] = SENT
    mlhs[1, :] = SENT
    for s in range(128 // N):
        mlhs[2 + s, :] = (SENT * (m_idx // N == s)).astype(NP_FP8)

    # static part of mask rhs rows 1..9 (per 128-column group)
    mrhs_static = np.zeros((MASK_K, 128), dtype=NP_FP8)
    mrhs_static[1, :] = -SENT
    for s in range(128 // N):
        mrhs_static[2 + s, :] = np.where(m_idx // N == s, SENT, np.float32(0.0)).astype(NP_FP8)

    def to_layout(a_core: np.ndarray) -> np.ndarray:
        # [rows=16384, 768] -> [g, n, k, p] -> [p, g, k, n]
        return np.ascontiguousarray(
            a_core.reshape(GROUPS, 128, KCH, 128).transpose(3, 0, 2, 1)
        ).reshape(128, GROUPS * KCH, 128)

    in_maps = []
    for c in range(N_CORES):
        r0 = c * ROWS_PER_CORE
        r1 = r0 + ROWS_PER_CORE
        mr = np.empty((MASK_K, ROWS_PER_CORE), dtype=NP_FP8)
        mr[0, :] = negc[r0:r1]
        mr[1:, :] = np.tile(mrhs_static[1:, :], (1, GROUPS))
        in_maps.append({
            "et": to_layout(Xn8[r0:r1]),
            "mrhs": mr,
            "mlhs": mlhs,
        })
    return in_maps


def kernel(embeddings: np.ndarray, labels: np.ndarray) -> np.ndarray:
    global LAST_RESULT
    assert embeddings.shape == (B, N, D)
    assert labels.shape == (B, N)

    X = np.asarray(embeddings, dtype=np.float32).reshape(ROWS, D)
    lab = np.asarray(labels).reshape(ROWS)

    ss = np.square(X).sum(axis=1, dtype=np.float32)
    norms = np.sqrt(ss)
    Xn8 = (X * (ESCALE / np.maximum(norms, np.float32(1e-12)))[:, None]).astype(NP_FP8)

    in_maps = _prep_core_inputs(Xn8, lab)

    if "nc" not in _CACHE:
        _CACHE["nc"] = _build_program()
    nc = _CACHE["nc"]

    trace = os.environ.get("BASS_KERNEL_TRACE", "0") == "1"
    res = run_bass_kernel_spmd(nc, in_maps, list(range(N_CORES)), trace=trace)
    LAST_RESULT = res

    # out[p, g]: group-major row r = g*128 + p
    maxneg = np.concatenate(
        [np.asarray(r["out"]).T.reshape(ROWS_PER_CORE) for r in res.results]
    )

    triplet = np.maximum(maxneg / SIM_SCALE + MARGIN, np.float32(0.0))
    has_neg = (np.asarray(labels) == 0).any(axis=1)
    w = (lab == 1) & np.repeat(has_neg, N)
    loss_sum = np.float32((triplet * w).sum(dtype=np.float64))
    count = int(w.sum())
    loss = np.float32(loss_sum / np.float32(max(count, 1)))
    return np.asarray(loss, dtype=np.float32)
